# revision 10
# baseline (speedup 1.0000x reference)
"""Trainium2 Bass kernel for LocalSelfAttention (sliding-window, causal).

Problem: val (S=4096, B=2, D=768); q/k/v projections then Longformer-style
banded causal attention, window = 256 lookback (keys j in [i-256, i]).

Sharding: 8 cores = batch (2) x sequence quarters (4). Each core handles
1024 queries of one batch element and receives a 256-row key/value halo
(recomputed locally from val rows; no inter-core communication).

Math simplifications (exact up to float rounding):
  - bk dropped: per-query additive constant q.bk cancels in softmax.
  - bv added on host at the end: sum_j p_j (v0_j + bv) = (PV)/Z + bv.
  - no max-subtraction in softmax: scores ~ N(0,1), |s| < ~8, exp is safe.
  - 1/sqrt(hd) folded into Wq/bq on host.
  - final division (PV / rowsum) done on host.

Device pipeline per core (all matmul inputs bf16):
  Phase A: v-projection first (token-stationary, scattered into per-head
  vaug tiles with a ones column for row sums), then per feature-tile m:
  q-projection (+bias via ACT) and k-projection, so attention head-pairs
  unlock progressively.
  Phase B: 24 pair-iterations (6 feature-tiles x 4 query groups of 256;
  each feature-tile holds 2 heads on partition halves 0-63 / 64-127 so
  the two heads' QK matmuls run concurrently on disjoint PE row-groups).
  Scores for a group live in a packed [128, 768] PSUM tile holding only
  the 3-key-tile band (kt1@0, kt2@256, kt0@512, kt3@640 - no matmul
  straddles a PSUM bank). One 768-col EXP per head, one DVE mask
  multiply, then PV with stationary = v-tile (65 cols incl. ones) and
  moving = probs, giving out[65, 256] per (head, group), DMA'd out
  transposed; the host untransposes and divides by the row sums.
"""

import os
import numpy as np
import ml_dtypes

S, B, D = 4096, 2, 768
H, HD = 12, 64
W = 256
NCORES = 8
SQ = S // 4            # 1024 queries per core
SKV = SQ + W           # 1280 kv rows (halo)
NG = SQ // 256         # 4 query groups of 256
ND = D // 128          # 6 feature tiles
NKVT = SKV // 128      # 10 kv tiles
VA = HD + 1            # 65: per-head v width incl. ones column
SCALE = 1.0 / np.sqrt(HD).astype(np.float32)  # 0.125

# packed scores layout: per 256-query group, 4 key tiles tau=0..3 with
# query windows [QLO, QLO+WID) packed at column OFF (bank-straddle free)
WID = (128, 256, 256, 128)
QLO = (0, 0, 0, 128)
OFF = (512, 0, 256, 640)

_CACHE = {}


def _masks_np(boundary: bool) -> np.ndarray:
    """(2, 128, 768) bf16 band masks in the packed scores layout.

    Partition = key-within-tile pp; columns = packed query windows.
    Set 0 is used for group 0 (kt0/kt1 zeroed on sequence-boundary cores),
    set 1 for groups 1..3.
    """
    pp = np.arange(128)[:, None]
    valid = [None] * 4
    j0 = np.arange(WID[0])[None, :]
    valid[0] = (pp >= j0)                 # keys g*256+pp, queries j
    j1 = np.arange(WID[1])[None, :]
    valid[1] = (j1 <= 128 + pp)
    j2 = np.arange(WID[2])[None, :]
    valid[2] = (j2 >= pp)
    j3 = np.arange(WID[3])[None, :]
    valid[3] = (j3 >= pp)

    def build(zero_lo: bool) -> np.ndarray:
        m = np.zeros((128, 768), np.float32)
        for tau in range(4):
            v = valid[tau].astype(np.float32)
            if zero_lo and tau in (0, 1):
                v = np.zeros_like(v)
            m[:, OFF[tau]:OFF[tau] + WID[tau]] = v
        return m

    m = np.stack([build(boundary), build(False)])
    m = np.concatenate([m, m], axis=2)   # duplicate for the head pair halves
    return np.ascontiguousarray(m.astype(ml_dtypes.bfloat16))


def _build_nc():
    import concourse.bacc as bacc
    import concourse.mybir as mybir
    from concourse.tile import TileContext

    f32 = mybir.dt.float32
    bf16 = mybir.dt.bfloat16
    AF = mybir.ActivationFunctionType

    nc = bacc.Bacc(trn_type="TRN2", debug=False, num_devices=NCORES)

    valT_d = nc.dram_tensor("valT", [D, SKV], bf16, kind="ExternalInput").ap()
    wq_d = nc.dram_tensor("wq", [D, D], bf16, kind="ExternalInput").ap()
    wk_d = nc.dram_tensor("wk", [D, D], bf16, kind="ExternalInput").ap()
    wv_d = nc.dram_tensor("wv", [D, D], bf16, kind="ExternalInput").ap()
    bq_d = nc.dram_tensor("bq", [D, 1], f32, kind="ExternalInput").ap()
    masks_d = nc.dram_tensor("masks", [2, 128, 1536], bf16, kind="ExternalInput").ap()
    out_d = nc.dram_tensor("out", [ND, NG, VA, 512], f32, kind="ExternalOutput").ap()

    with TileContext(nc) as tc:
        with tc.tile_pool(name="persist", bufs=1) as pp, \
             tc.tile_pool(name="stage", bufs=1) as sp, \
             tc.tile_pool(name="pa512", bufs=3, space="PSUM") as pps, \
             tc.tile_pool(name="probsp", bufs=10) as prp, \
             tc.tile_pool(name="scps", bufs=2, space="PSUM") as scp, \
             tc.tile_pool(name="pvps", bufs=1, space="PSUM") as pvp, \
             tc.tile_pool(name="outp", bufs=3) as outp:
            qT = [pp.tile([128, SQ], bf16, name=f"qT{m}", tag=f"qT{m}") for m in range(ND)]
            kT = [pp.tile([128, SKV], bf16, name=f"kT{m}", tag=f"kT{m}") for m in range(ND)]
            vaug = [pp.tile([128, H * VA], bf16, name=f"vaug{t}", tag=f"vaug{t}") for t in range(NKVT)]
            bqt = [pp.tile([128, 1], f32, name=f"bqt{m}", tag=f"bqt{m}") for m in range(ND)]
            maskt = [pp.tile([128, 1536], bf16, name=f"maskt{i}", tag=f"maskt{i}") for i in range(2)]

            # ---- input DMAs. Pairwise (valT[k], w[k]) ordering so the
            # first projection groups unblock as tiles land; round-robin
            # over the three DMA-capable engine queues.
            engines = [nc.sync, nc.scalar, nc.gpsimd]
            valT_t = [sp.tile([128, SKV], bf16, name=f"valTt{k}", tag=f"valTt{k}") for k in range(ND)]
            wv_t = [sp.tile([128, D], bf16, name=f"wvt{k}", tag=f"wvt{k}") for k in range(ND)]
            wq_t = [sp.tile([128, D], bf16, name=f"wqt{k}", tag=f"wqt{k}") for k in range(ND)]
            wk_t = [sp.tile([128, D], bf16, name=f"wkt{k}", tag=f"wkt{k}") for k in range(ND)]
            ei = 0
            def dma(dst, src):
                nonlocal ei
                engines[ei % 3].dma_start(dst, src)
                ei += 1
            # prelude (q0/k0) inputs first, rotated across queues: the
            # 2-per-iteration emission keeps each tensor type off a single
            # queue so the k-th accumulation matmul unblocks progressively
            for k in range(ND):
                dma(wq_t[k][:], wq_d[k * 128:(k + 1) * 128, :])
                dma(valT_t[k][:, 0:640], valT_d[k * 128:(k + 1) * 128, 0:640])
            for k in range(ND):
                dma(wk_t[k][:], wk_d[k * 128:(k + 1) * 128, :])
            for k in range(ND):
                dma(valT_t[k][:, 640:SKV], valT_d[k * 128:(k + 1) * 128, 640:SKV])
                dma(wv_t[k][:], wv_d[k * 128:(k + 1) * 128, :])
            for m in range(ND):
                nc.gpsimd.dma_start(bqt[m][:], bq_d[m * 128:(m + 1) * 128, :])
            for i in range(2):
                dma(maskt[i][:], masks_d[i])

            for t in range(NKVT):
                ones_col = vaug[t][:].rearrange("p (h c) -> p h c", c=VA)[:, :, HD:VA]
                nc.vector.memset(ones_col, 1.0)

            # ---------------- Phase A chunks (one PSUM group each) --------
            def emit_vproj(t, s):
                ps = pps.tile([128, 512], f32, name="psv", tag="pa")
                lo, hi, h0 = (0, 512, 0) if s == 0 else (512, 768, 8)
                w = hi - lo
                for k in range(ND):
                    nc.tensor.matmul(ps[:, 0:w],
                                     valT_t[k][:, t * 128:(t + 1) * 128],
                                     wv_t[k][:, lo:hi],
                                     start=(k == 0), stop=(k == ND - 1))
                nh = w // HD
                va = vaug[t][:].rearrange("p (h c) -> p h c", c=VA)
                nc.vector.tensor_copy(
                    va[:, h0:h0 + nh, 0:HD],
                    ps[:, 0:w].rearrange("p (h c) -> p h c", c=HD))

            def emit_qproj(m, ch):
                ps = pps.tile([128, 512], f32, name="psq", tag="pa")
                for k in range(ND):
                    nc.tensor.matmul(ps[:],
                                     wq_t[k][:, m * 128:(m + 1) * 128],
                                     valT_t[k][:, W + ch * 512:W + (ch + 1) * 512],
                                     start=(k == 0), stop=(k == ND - 1))
                nc.scalar.activation(qT[m][:, ch * 512:(ch + 1) * 512], ps[:],
                                     AF.Identity, bias=bqt[m][:], scale=1.0)

            def emit_kproj(m, s):
                ps = pps.tile([128, 512], f32, name="psk", tag="pa")
                lo, hi = (s * 512, min((s + 1) * 512, SKV))
                w = hi - lo
                for k in range(ND):
                    nc.tensor.matmul(ps[:, 0:w],
                                     wk_t[k][:, m * 128:(m + 1) * 128],
                                     valT_t[k][:, lo:hi],
                                     start=(k == 0), stop=(k == ND - 1))
                nc.vector.tensor_copy(kT[m][:, lo:hi], ps[:, 0:w])

            # ---------------- Phase B ----------------
            pair_iters = [(mh, g) for mh in range(ND) for g in range(NG)]
            probs_ring = {}

            def emit_qk_side(i):
                mh, g = pair_iters[i]
                pss = [scp.tile([128, 768], f32, name="pss", tag="scores")
                       for _ in range(2)]
                for tau in range(4):
                    for hh in range(2):
                        ph = hh * 64
                        nc.tensor.matmul(
                            pss[hh][:, OFF[tau]:OFF[tau] + WID[tau]],
                            kT[mh][ph:ph + 64, (2 * g + tau) * 128:(2 * g + tau + 1) * 128],
                            qT[mh][ph:ph + 64, g * 256 + QLO[tau]:g * 256 + QLO[tau] + WID[tau]],
                            start=True, stop=True)
                pre = prp.tile([128, 1536], bf16, name="prexp", tag="prexp")
                for hh in range(2):
                    nc.scalar.activation(pre[:, hh * 768:(hh + 1) * 768], pss[hh][:], AF.Exp)
                pr = prp.tile([128, 1536], bf16, name="probs", tag="probs")
                mt = maskt[0] if g == 0 else maskt[1]
                nc.vector.tensor_mul(pr[:], pre[:], mt[:])
                probs_ring[i] = pr

            def emit_pv_side(i):
                mh, g = pair_iters[i]
                pr = probs_ring.pop(i)
                pso = pvp.tile([VA, 512], f32, name="pso", tag="pv")
                for hh in range(2):
                    h = 2 * mh + hh
                    for tau in range(4):
                        nc.tensor.matmul(
                            pso[0:VA, hh * 256 + QLO[tau]:hh * 256 + QLO[tau] + WID[tau]],
                            vaug[2 * g + tau][:, h * VA:(h + 1) * VA],
                            pr[:, hh * 768 + OFF[tau]:hh * 768 + OFF[tau] + WID[tau]],
                            start=(hh == 0 and tau == 0), stop=(hh == 1 and tau == 3))
                ob = outp.tile([VA, 512], f32, name="ob", tag="ob")
                nc.vector.tensor_copy(ob[:], pso[:])
                nc.sync.dma_start(out_d[mh, g], ob[:])

            # ------------- just-in-time interleaved emission -------------
            # tapered PV lag: deep early (so v-projection can trickle in as
            # spacers) and shallow late (short drain tail)
            NPI = len(pair_iters)
            def lag(j):
                return max(2, 8 - j // 2)
            pv_slot = {}
            leftover = []
            for j in range(NPI):
                sl = j + lag(j)
                if sl <= NPI - 1:
                    pv_slot.setdefault(sl, []).append(j)
                else:
                    leftover.append(j)

            # chunks with deadlines (must be emitted before QK/PV slot d)
            chunks = []   # (deadline, emit_fn)
            for m in range(ND):
                for ch in range(2):
                    if (m, ch) == (0, 0):
                        continue          # prelude
                    chunks.append((4 * m + 2 * ch, lambda m=m, ch=ch: emit_qproj(m, ch)))
            for m in range(ND):
                for sn in range(3):
                    if m == 0 and sn < 2:
                        continue          # prelude
                    d = 4 * m if sn < 2 else 4 * m + 2
                    chunks.append((d, lambda m=m, sn=sn: emit_kproj(m, sn)))
            first_pv_slot = {}
            for slot, js in pv_slot.items():
                for j in js:
                    for t in range(2 * (j % NG), 2 * (j % NG) + 4):
                        if t not in first_pv_slot:
                            first_pv_slot[t] = slot
            for t in range(NKVT):
                d = first_pv_slot.get(t, 8)
                for sn in range(2):
                    chunks.append((d, lambda t=t, sn=sn: emit_vproj(t, sn)))
            chunks.sort(key=lambda c: c[0])

            # prelude: just enough for QK (0,0) and (0,1)
            emit_qproj(0, 0)
            emit_kproj(0, 0)
            emit_kproj(0, 1)

            emitted = 0
            total = len(chunks)
            for i in range(NPI):
                # overdue chunks first
                while emitted < total and chunks[emitted][0] <= i:
                    chunks[emitted][1]()
                    emitted += 1
                emit_qk_side(i)
                # spread remaining chunks evenly across iterations
                target = (i + 1) * total // NPI
                while emitted < min(target, total):
                    chunks[emitted][1]()
                    emitted += 1
                for j in pv_slot.get(i, []):
                    emit_pv_side(j)
            while emitted < total:
                chunks[emitted][1]()
                emitted += 1
            for j in leftover:
                emit_pv_side(j)
    nc.compile()
    return nc


def _get_nc():
    if "nc" not in _CACHE:
        _CACHE["nc"] = _build_nc()
    return _CACHE["nc"]


def _install_ntff_hook():
    """Provide antenv.axon_hooks (absent in this image) so bass_utils can
    NTFF-profile under axon, using trn_agent_boot's ctypes hook builder."""
    import sys
    import types
    try:
        from antenv.axon_hooks import get_axon_ntff_profile_hook  # noqa: F401
        return
    except ImportError:
        pass
    try:
        import antenv
        from trn_agent_boot.trn_boot import _ntff_profile_via_ctypes
        hook = _ntff_profile_via_ctypes("/opt/axon/libaxon_pjrt.so")
        mod = types.ModuleType("antenv.axon_hooks")
        mod.get_axon_ntff_profile_hook = lambda: hook
        mod.set_axon_ntff_profile_hook = lambda h: None
        sys.modules["antenv.axon_hooks"] = mod
        antenv.axon_hooks = mod
    except Exception as e:  # profiling is best-effort
        print(f"ntff hook install failed: {e}")


def kernel(val, Wq, bq, Wk, bk, Wv, bv):
    from concourse.bass_utils import run_bass_kernel_spmd

    val = np.asarray(val, dtype=np.float32)
    Wq = np.asarray(Wq, dtype=np.float32)
    bq = np.asarray(bq, dtype=np.float32)
    Wk = np.asarray(Wk, dtype=np.float32)
    Wv = np.asarray(Wv, dtype=np.float32)
    bv = np.asarray(bv, dtype=np.float32)

    bf = ml_dtypes.bfloat16
    wq_s = np.ascontiguousarray((Wq * SCALE).astype(bf))
    bq_s = np.ascontiguousarray((bq * SCALE).reshape(D, 1))
    wk_c = np.ascontiguousarray(Wk.astype(bf))
    wv_c = np.ascontiguousarray(Wv.astype(bf))

    in_maps = []
    for c in range(NCORES):
        b, qd = divmod(c, 4)
        lo = qd * SQ - W
        hi = qd * SQ + SQ
        vs = val[max(lo, 0):hi, b, :]
        if lo < 0:
            vs = np.concatenate([np.zeros((-lo, D), np.float32), vs], axis=0)
        in_maps.append({
            "valT": np.ascontiguousarray(vs.T.astype(bf)),
            "wq": wq_s, "wk": wk_c, "wv": wv_c, "bq": bq_s,
            "masks": _masks_np(boundary=(qd == 0)),
        })

    nc = _get_nc()
    trace = os.environ.get("BASS_KERNEL_TRACE", "0") == "1"
    kwargs = {}
    if trace:
        _install_ntff_hook()
        kwargs = dict(trace=True, tmpdir=os.environ.get("BASS_KERNEL_TRACE_DIR") or None)
    res = run_bass_kernel_spmd(nc, in_maps, list(range(NCORES)), **kwargs)
    _CACHE["last_result"] = res

    out = np.empty((S, B, D), np.float32)
    for c in range(NCORES):
        b, qd = divmod(c, 4)
        raw = np.asarray(res.results[c]["out"]).reshape(ND, NG, VA, 2, 256)
        core = raw[:, :, 0:HD] / raw[:, :, HD:VA]            # (ND, NG, HD, 2, 256)
        # (mh, g, d, hh, q) -> (g, q, mh, hh, d) -> (SQ, D)
        core = core.transpose(1, 4, 0, 3, 2).reshape(SQ, D)
        out[qd * SQ:(qd + 1) * SQ, b, :] = core
    out += bv
    return out


# revision 15
# speedup vs baseline: 1.1403x; 1.1403x over previous
"""Trainium2 Bass kernel for LocalSelfAttention (sliding-window, causal).

Problem: val (S=4096, B=2, D=768); q/k/v projections then Longformer-style
banded causal attention, window = 256 lookback (keys j in [i-256, i]).

Sharding: 8 cores = batch (2) x sequence quarters (4). Each core handles
1024 queries of one batch element and receives a 256-row key/value halo
(recomputed locally from val rows; no inter-core communication).

Math simplifications (exact up to float rounding):
  - bk dropped: per-query additive constant q.bk cancels in softmax.
  - bv added on host at the end: sum_j p_j (v0_j + bv) = (PV)/Z + bv.
  - no max-subtraction in softmax: scores ~ N(0,1), |s| < ~8, exp is safe.
  - 1/sqrt(hd) folded into Wq/bq on host.
  - final division (PV / rowsum) done on host.

Device pipeline per core (all matmul inputs bf16):
  One fused software-pipelined schedule. A small prelude (q-projection
  feature-tile 0, k-projection tile 0) unblocks attention within ~15us;
  every remaining projection accumulation group (q/k/v per feature- or
  token-tile) is a "chunk" with a deadline, emitted just-in-time between
  attention iterations so the PE stays >90% busy end to end. Input DMAs
  are issued in two groups (prelude payloads first) because each
  dma_start costs ~0.8us of descriptor generation on its engine.

  Attention runs as 24 pair-iterations (6 feature-tiles x 4 query
  groups of 256; a feature-tile holds 2 heads on partition halves
  0-63 / 64-127, and their QK matmuls alternate so they run
  concurrently on disjoint PE row-groups). Scores live in a packed
  [128, 768] PSUM tile holding only the 3-key-tile band (kt1@0,
  kt2@256, kt0@512, kt3@640 - no matmul output straddles a PSUM bank).
  One 768-col EXP per head writes both halves of a [128, 1536] pair
  tile, one DVE multiply applies the band mask for both heads, then PV
  uses stationary = v-tile (65 cols incl. a ones column for row sums)
  and moving = probs, accumulating out[65, 512] per (head-pair, group)
  in a single PSUM bank - the staggered query windows compose via the
  per-element has_written accumulate bits. The PV side trails the QK
  side by a tapered lag (deep early, so v-projection chunks can
  trickle in; shallow late, for a short drain tail). Output tiles are
  DMA'd transposed; the host untransposes and divides by the row sums.
"""

import os
import numpy as np
import ml_dtypes

S, B, D = 4096, 2, 768
H, HD = 12, 64
W = 256
NCORES = 8
SQ = S // 4            # 1024 queries per core
SKV = SQ + W           # 1280 kv rows (halo)
NG = SQ // 256         # 4 query groups of 256
ND = D // 128          # 6 feature tiles
NKVT = SKV // 128      # 10 kv tiles
VA = HD + 1            # 65: per-head v width incl. ones column
SCALE = 1.0 / np.sqrt(HD).astype(np.float32)  # 0.125

# packed scores layout: per 256-query group, 4 key tiles tau=0..3 with
# query windows [QLO, QLO+WID) packed at column OFF (bank-straddle free)
WID = (128, 256, 256, 128)
QLO = (0, 0, 0, 128)
OFF = (512, 0, 256, 640)

_CACHE = {}


def _masks_np(boundary: bool) -> np.ndarray:
    """(2, 128, 768) bf16 band masks in the packed scores layout.

    Partition = key-within-tile pp; columns = packed query windows.
    Set 0 is used for group 0 (kt0/kt1 zeroed on sequence-boundary cores),
    set 1 for groups 1..3.
    """
    pp = np.arange(128)[:, None]
    valid = [None] * 4
    j0 = np.arange(WID[0])[None, :]
    valid[0] = (pp >= j0)                 # keys g*256+pp, queries j
    j1 = np.arange(WID[1])[None, :]
    valid[1] = (j1 <= 128 + pp)
    j2 = np.arange(WID[2])[None, :]
    valid[2] = (j2 >= pp)
    j3 = np.arange(WID[3])[None, :]
    valid[3] = (j3 >= pp)

    def build(zero_lo: bool) -> np.ndarray:
        m = np.zeros((128, 768), np.float32)
        for tau in range(4):
            v = valid[tau].astype(np.float32)
            if zero_lo and tau in (0, 1):
                v = np.zeros_like(v)
            m[:, OFF[tau]:OFF[tau] + WID[tau]] = v
        return m

    m = np.stack([build(boundary), build(False)])
    m = np.concatenate([m, m], axis=2)   # duplicate for the head pair halves
    return np.ascontiguousarray(m.astype(ml_dtypes.bfloat16))


def _build_nc():
    import concourse.bacc as bacc
    import concourse.mybir as mybir
    from concourse.tile import TileContext

    f32 = mybir.dt.float32
    bf16 = mybir.dt.bfloat16
    AF = mybir.ActivationFunctionType

    nc = bacc.Bacc(trn_type="TRN2", debug=False, num_devices=NCORES)

    valT_d = nc.dram_tensor("valT", [D, SKV], bf16, kind="ExternalInput").ap()
    wq_d = nc.dram_tensor("wq", [D, D], bf16, kind="ExternalInput").ap()
    wk_d = nc.dram_tensor("wk", [D, D], bf16, kind="ExternalInput").ap()
    wv_d = nc.dram_tensor("wv", [D, D], bf16, kind="ExternalInput").ap()
    bq_d = nc.dram_tensor("bq", [D, 1], f32, kind="ExternalInput").ap()
    masks_d = nc.dram_tensor("masks", [2, 128, 1536], bf16, kind="ExternalInput").ap()
    out_d = nc.dram_tensor("out", [ND, NG, VA, 512], f32, kind="ExternalOutput").ap()

    with TileContext(nc) as tc:
        with tc.tile_pool(name="persist", bufs=1) as pp, \
             tc.tile_pool(name="stage", bufs=1) as sp, \
             tc.tile_pool(name="pa512", bufs=3, space="PSUM") as pps, \
             tc.tile_pool(name="probsp", bufs=10) as prp, \
             tc.tile_pool(name="scps", bufs=2, space="PSUM") as scp, \
             tc.tile_pool(name="pvps", bufs=1, space="PSUM") as pvp, \
             tc.tile_pool(name="outp", bufs=3) as outp:
            qT = [pp.tile([128, SQ], bf16, name=f"qT{m}", tag=f"qT{m}") for m in range(ND)]
            kT = [pp.tile([128, SKV], bf16, name=f"kT{m}", tag=f"kT{m}") for m in range(ND)]
            vaug = [pp.tile([128, H * VA], bf16, name=f"vaug{t}", tag=f"vaug{t}") for t in range(NKVT)]

            # ---- staged input tiles; DMAs split into two issue groups:
            # dma_start costs ~0.8us of descriptor generation on the
            # issuing engine, so only the prelude-critical DMAs go first
            # and the rest are emitted after the first compute chunk.
            engines = [nc.sync, nc.scalar, nc.gpsimd]
            valT_t = [sp.tile([128, SKV], bf16, name=f"valTt{k}", tag=f"valTt{k}") for k in range(ND)]
            wv_t = [sp.tile([128, D], bf16, name=f"wvt{k}", tag=f"wvt{k}") for k in range(ND)]
            wq_t = [sp.tile([128, D], bf16, name=f"wqt{k}", tag=f"wqt{k}") for k in range(ND)]
            wk_t = [sp.tile([128, D], bf16, name=f"wkt{k}", tag=f"wkt{k}") for k in range(ND)]
            bqt = [sp.tile([128, 1], f32, name=f"bqt{m}", tag=f"bqt{m}") for m in range(ND)]
            maskt = [sp.tile([128, 1536], bf16, name=f"maskt{i}", tag=f"maskt{i}") for i in range(2)]
            ei = 0
            def dma(dst, src):
                nonlocal ei
                engines[ei % 3].dma_start(dst, src)
                ei += 1

            def emit_dma_group_a():
                # sync+scalar alternate the prelude payloads; gpsimd takes
                # the tiny bq vectors (q(0,0)'s bias evac reads them)
                eng2 = [nc.sync, nc.scalar]
                for k in range(ND):
                    eng2[k % 2].dma_start(wq_t[k][:], wq_d[k * 128:(k + 1) * 128, :])
                    eng2[(k + 1) % 2].dma_start(valT_t[k][:, 0:768], valT_d[k * 128:(k + 1) * 128, 0:768])
                for m in range(ND):
                    nc.gpsimd.dma_start(bqt[m][:], bq_d[m * 128:(m + 1) * 128, :])

            def emit_dma_group_b():
                # sync: valT tails + masks; gpsimd: wk + wv. The scalar
                # queue stays clear: it runs the q-bias/exp activations.
                for k in range(ND):
                    nc.gpsimd.dma_start(wk_t[k][:], wk_d[k * 128:(k + 1) * 128, :])
                    nc.sync.dma_start(valT_t[k][:, 768:SKV], valT_d[k * 128:(k + 1) * 128, 768:SKV])
                for i in range(2):
                    nc.sync.dma_start(maskt[i][:], masks_d[i])
                for k in range(ND):
                    nc.gpsimd.dma_start(wv_t[k][:], wv_d[k * 128:(k + 1) * 128, :])

            for t in range(NKVT):
                ones_col = vaug[t][:].rearrange("p (h c) -> p h c", c=VA)[:, :, HD:VA]
                nc.vector.memset(ones_col, 1.0)

            # ---------------- Phase A chunks (one PSUM group each) --------
            def emit_vproj(t, s):
                ps = pps.tile([128, 512], f32, name="psv", tag="pa")
                lo, hi, h0 = (0, 512, 0) if s == 0 else (512, 768, 8)
                w = hi - lo
                for k in range(ND):
                    nc.tensor.matmul(ps[:, 0:w],
                                     valT_t[k][:, t * 128:(t + 1) * 128],
                                     wv_t[k][:, lo:hi],
                                     start=(k == 0), stop=(k == ND - 1))
                nh = w // HD
                va = vaug[t][:].rearrange("p (h c) -> p h c", c=VA)
                nc.vector.tensor_copy(
                    va[:, h0:h0 + nh, 0:HD],
                    ps[:, 0:w].rearrange("p (h c) -> p h c", c=HD))

            def emit_qproj(m, ch):
                ps = pps.tile([128, 512], f32, name="psq", tag="pa")
                for k in range(ND):
                    nc.tensor.matmul(ps[:],
                                     wq_t[k][:, m * 128:(m + 1) * 128],
                                     valT_t[k][:, W + ch * 512:W + (ch + 1) * 512],
                                     start=(k == 0), stop=(k == ND - 1))
                nc.scalar.activation(qT[m][:, ch * 512:(ch + 1) * 512], ps[:],
                                     AF.Identity, bias=bqt[m][:], scale=1.0)

            def emit_kproj(m, s):
                ps = pps.tile([128, 512], f32, name="psk", tag="pa")
                lo, hi = (s * 512, min((s + 1) * 512, SKV))
                w = hi - lo
                for k in range(ND):
                    nc.tensor.matmul(ps[:, 0:w],
                                     wk_t[k][:, m * 128:(m + 1) * 128],
                                     valT_t[k][:, lo:hi],
                                     start=(k == 0), stop=(k == ND - 1))
                nc.vector.tensor_copy(kT[m][:, lo:hi], ps[:, 0:w])

            # ---------------- Phase B ----------------
            pair_iters = [(mh, g) for mh in range(ND) for g in range(NG)]
            probs_ring = {}

            def emit_qk_side(i):
                mh, g = pair_iters[i]
                pss = [scp.tile([128, 768], f32, name="pss", tag="scores")
                       for _ in range(2)]
                for tau in range(4):
                    for hh in range(2):
                        ph = hh * 64
                        nc.tensor.matmul(
                            pss[hh][:, OFF[tau]:OFF[tau] + WID[tau]],
                            kT[mh][ph:ph + 64, (2 * g + tau) * 128:(2 * g + tau + 1) * 128],
                            qT[mh][ph:ph + 64, g * 256 + QLO[tau]:g * 256 + QLO[tau] + WID[tau]],
                            start=True, stop=True)
                pre = prp.tile([128, 1536], bf16, name="prexp", tag="prexp")
                for hh in range(2):
                    nc.scalar.activation(pre[:, hh * 768:(hh + 1) * 768], pss[hh][:], AF.Exp)
                pr = prp.tile([128, 1536], bf16, name="probs", tag="probs")
                mt = maskt[0] if g == 0 else maskt[1]
                nc.vector.tensor_mul(pr[:], pre[:], mt[:])
                probs_ring[i] = pr

            def emit_pv_side(i):
                mh, g = pair_iters[i]
                pr = probs_ring.pop(i)
                pso = pvp.tile([VA, 512], f32, name="pso", tag="pv")
                for hh in range(2):
                    h = 2 * mh + hh
                    for tau in range(4):
                        nc.tensor.matmul(
                            pso[0:VA, hh * 256 + QLO[tau]:hh * 256 + QLO[tau] + WID[tau]],
                            vaug[2 * g + tau][:, h * VA:(h + 1) * VA],
                            pr[:, hh * 768 + OFF[tau]:hh * 768 + OFF[tau] + WID[tau]],
                            start=(hh == 0 and tau == 0), stop=(hh == 1 and tau == 3))
                ob = outp.tile([VA, 512], f32, name="ob", tag="ob")
                nc.vector.tensor_copy(ob[:], pso[:])
                nc.sync.dma_start(out_d[mh, g], ob[:])

            # ------------- just-in-time interleaved emission -------------
            # tapered PV lag: deep early (so v-projection can trickle in as
            # spacers) and shallow late (short drain tail)
            NPI = len(pair_iters)
            def lag(j):
                return max(2, 8 - j // 2)
            pv_slot = {}
            leftover = []
            for j in range(NPI):
                sl = j + lag(j)
                if sl <= NPI - 1:
                    pv_slot.setdefault(sl, []).append(j)
                else:
                    leftover.append(j)

            # chunks with deadlines (must be emitted before QK/PV slot d)
            chunks = []   # (deadline, emit_fn)
            for m in range(ND):
                for ch in range(2):
                    if (m, ch) == (0, 0):
                        continue          # prelude
                    chunks.append((4 * m + 2 * ch, lambda m=m, ch=ch: emit_qproj(m, ch)))
            for m in range(ND):
                for sn in range(3):
                    d = 4 * m if sn < 2 else 4 * m + 2
                    chunks.append((d, lambda m=m, sn=sn: emit_kproj(m, sn)))
            first_pv_slot = {}
            for slot, js in pv_slot.items():
                for j in js:
                    for t in range(2 * (j % NG), 2 * (j % NG) + 4):
                        if t not in first_pv_slot:
                            first_pv_slot[t] = slot
            for t in range(NKVT):
                d = first_pv_slot.get(t, 8)
                for sn in range(2):
                    chunks.append((d, lambda t=t, sn=sn: emit_vproj(t, sn)))
            chunks.sort(key=lambda c: c[0])

            # prelude: group-A DMAs, first q chunk, then the rest of the
            # input DMAs (k(0,s0/s1) are deadline-0 chunks below)
            emit_dma_group_a()
            emit_qproj(0, 0)
            emit_dma_group_b()

            emitted = 0
            total = len(chunks)
            for i in range(NPI):
                # overdue chunks first
                while emitted < total and chunks[emitted][0] <= i:
                    chunks[emitted][1]()
                    emitted += 1
                emit_qk_side(i)
                # spread remaining chunks evenly across iterations
                target = (i + 1) * total // NPI
                while emitted < min(target, total):
                    chunks[emitted][1]()
                    emitted += 1
                for j in pv_slot.get(i, []):
                    emit_pv_side(j)
            while emitted < total:
                chunks[emitted][1]()
                emitted += 1
            for j in leftover:
                emit_pv_side(j)
    nc.compile()
    return nc


def _get_nc():
    if "nc" not in _CACHE:
        _CACHE["nc"] = _build_nc()
    return _CACHE["nc"]


def _install_ntff_hook():
    """Provide antenv.axon_hooks (absent in this image) so bass_utils can
    NTFF-profile under axon, using trn_agent_boot's ctypes hook builder."""
    import sys
    import types
    try:
        from antenv.axon_hooks import get_axon_ntff_profile_hook  # noqa: F401
        return
    except ImportError:
        pass
    try:
        import antenv
        from trn_agent_boot.trn_boot import _ntff_profile_via_ctypes
        hook = _ntff_profile_via_ctypes("/opt/axon/libaxon_pjrt.so")
        mod = types.ModuleType("antenv.axon_hooks")
        mod.get_axon_ntff_profile_hook = lambda: hook
        mod.set_axon_ntff_profile_hook = lambda h: None
        sys.modules["antenv.axon_hooks"] = mod
        antenv.axon_hooks = mod
    except Exception as e:  # profiling is best-effort
        print(f"ntff hook install failed: {e}")


def kernel(val, Wq, bq, Wk, bk, Wv, bv):
    from concourse.bass_utils import run_bass_kernel_spmd

    val = np.asarray(val, dtype=np.float32)
    Wq = np.asarray(Wq, dtype=np.float32)
    bq = np.asarray(bq, dtype=np.float32)
    Wk = np.asarray(Wk, dtype=np.float32)
    Wv = np.asarray(Wv, dtype=np.float32)
    bv = np.asarray(bv, dtype=np.float32)

    bf = ml_dtypes.bfloat16
    wq_s = np.ascontiguousarray((Wq * SCALE).astype(bf))
    bq_s = np.ascontiguousarray((bq * SCALE).reshape(D, 1))
    wk_c = np.ascontiguousarray(Wk.astype(bf))
    wv_c = np.ascontiguousarray(Wv.astype(bf))

    in_maps = []
    for c in range(NCORES):
        b, qd = divmod(c, 4)
        lo = qd * SQ - W
        hi = qd * SQ + SQ
        vs = val[max(lo, 0):hi, b, :]
        if lo < 0:
            vs = np.concatenate([np.zeros((-lo, D), np.float32), vs], axis=0)
        in_maps.append({
            "valT": np.ascontiguousarray(vs.T.astype(bf)),
            "wq": wq_s, "wk": wk_c, "wv": wv_c, "bq": bq_s,
            "masks": _masks_np(boundary=(qd == 0)),
        })

    nc = _get_nc()
    trace = os.environ.get("BASS_KERNEL_TRACE", "0") == "1"
    kwargs = {}
    if trace:
        _install_ntff_hook()
        kwargs = dict(trace=True, tmpdir=os.environ.get("BASS_KERNEL_TRACE_DIR") or None)
    res = run_bass_kernel_spmd(nc, in_maps, list(range(NCORES)), **kwargs)
    _CACHE["last_result"] = res

    out = np.empty((S, B, D), np.float32)
    for c in range(NCORES):
        b, qd = divmod(c, 4)
        raw = np.asarray(res.results[c]["out"]).reshape(ND, NG, VA, 2, 256)
        core = raw[:, :, 0:HD] / raw[:, :, HD:VA]            # (ND, NG, HD, 2, 256)
        # (mh, g, d, hh, q) -> (g, q, mh, hh, d) -> (SQ, D)
        core = core.transpose(1, 4, 0, 3, 2).reshape(SQ, D)
        out[qd * SQ:(qd + 1) * SQ, b, :] = core
    out += bv
    return out


# revision 16
# speedup vs baseline: 1.1568x; 1.0144x over previous
"""Trainium2 Bass kernel for LocalSelfAttention (sliding-window, causal).

Problem: val (S=4096, B=2, D=768); q/k/v projections then Longformer-style
banded causal attention, window = 256 lookback (keys j in [i-256, i]).

Sharding: 8 cores = batch (2) x sequence quarters (4). Each core handles
1024 queries of one batch element and receives a 256-row key/value halo
(recomputed locally from val rows; no inter-core communication).

Math simplifications (exact up to float rounding):
  - bk dropped: per-query additive constant q.bk cancels in softmax.
  - bv added on host at the end: sum_j p_j (v0_j + bv) = (PV)/Z + bv.
  - no max-subtraction in softmax: scores ~ N(0,1), |s| < ~8, exp is safe.
  - 1/sqrt(hd) folded into Wq/bq on host.
  - final division (PV / rowsum) done on host.

Device pipeline per core (all matmul inputs bf16):
  One fused software-pipelined schedule. A small prelude (q-projection
  feature-tile 0, k-projection tile 0) unblocks attention within ~15us;
  every remaining projection accumulation group (q/k/v per feature- or
  token-tile) is a "chunk" with a deadline, emitted just-in-time between
  attention iterations so the PE stays >90% busy end to end. Input DMAs
  are issued in two groups (prelude payloads first) because each
  dma_start costs ~0.8us of descriptor generation on its engine.

  Attention runs as 24 pair-iterations (6 feature-tiles x 4 query
  groups of 256; a feature-tile holds 2 heads on partition halves
  0-63 / 64-127, and their QK matmuls alternate so they run
  concurrently on disjoint PE row-groups). Scores live in a packed
  [128, 768] PSUM tile holding only the 3-key-tile band (kt1@0,
  kt2@256, kt0@512, kt3@640 - no matmul output straddles a PSUM bank).
  One 768-col EXP per head writes both halves of a [128, 1536] pair
  tile, one DVE multiply applies the band mask for both heads, then PV
  uses stationary = v-tile (65 cols incl. a ones column for row sums)
  and moving = probs, accumulating out[65, 512] per (head-pair, group)
  in a single PSUM bank - the staggered query windows compose via the
  per-element has_written accumulate bits. The PV side trails the QK
  side by a tapered lag (deep early, so v-projection chunks can
  trickle in; shallow late, for a short drain tail). Output tiles are
  DMA'd transposed; the host untransposes and divides by the row sums.
"""

import os
import numpy as np
import ml_dtypes

S, B, D = 4096, 2, 768
H, HD = 12, 64
W = 256
NCORES = 8
SQ = S // 4            # 1024 queries per core
SKV = SQ + W           # 1280 kv rows (halo)
NG = SQ // 256         # 4 query groups of 256
ND = D // 128          # 6 feature tiles
NKVT = SKV // 128      # 10 kv tiles
VA = HD + 1            # 65: per-head v width incl. ones column
SCALE = 1.0 / np.sqrt(HD).astype(np.float32)  # 0.125

# packed scores layout: per 256-query group, 4 key tiles tau=0..3 with
# query windows [QLO, QLO+WID) packed at column OFF (bank-straddle free)
WID = (128, 256, 256, 128)
QLO = (0, 0, 0, 128)
OFF = (512, 0, 256, 640)

_CACHE = {}


def _masks_np(boundary: bool) -> np.ndarray:
    """(2, 128, 768) bf16 band masks in the packed scores layout.

    Partition = key-within-tile pp; columns = packed query windows.
    Set 0 is used for group 0 (kt0/kt1 zeroed on sequence-boundary cores),
    set 1 for groups 1..3.
    """
    pp = np.arange(128)[:, None]
    valid = [None] * 4
    j0 = np.arange(WID[0])[None, :]
    valid[0] = (pp >= j0)                 # keys g*256+pp, queries j
    j1 = np.arange(WID[1])[None, :]
    valid[1] = (j1 <= 128 + pp)
    j2 = np.arange(WID[2])[None, :]
    valid[2] = (j2 >= pp)
    j3 = np.arange(WID[3])[None, :]
    valid[3] = (j3 >= pp)

    def build(zero_lo: bool) -> np.ndarray:
        m = np.zeros((128, 768), np.float32)
        for tau in range(4):
            v = valid[tau].astype(np.float32)
            if zero_lo and tau in (0, 1):
                v = np.zeros_like(v)
            m[:, OFF[tau]:OFF[tau] + WID[tau]] = v
        return m

    m = np.stack([build(boundary), build(False)])
    m = np.concatenate([m, m], axis=2)   # duplicate for the head pair halves
    return np.ascontiguousarray(m.astype(ml_dtypes.bfloat16))


def _build_nc():
    import concourse.bacc as bacc
    import concourse.mybir as mybir
    from concourse.tile import TileContext

    f32 = mybir.dt.float32
    bf16 = mybir.dt.bfloat16
    AF = mybir.ActivationFunctionType

    nc = bacc.Bacc(trn_type="TRN2", debug=False, num_devices=NCORES)

    valT_d = nc.dram_tensor("valT", [D, SKV], bf16, kind="ExternalInput").ap()
    wq_d = nc.dram_tensor("wq", [D, D], bf16, kind="ExternalInput").ap()
    wk_d = nc.dram_tensor("wk", [D, D], bf16, kind="ExternalInput").ap()
    wv_d = nc.dram_tensor("wv", [D, D], bf16, kind="ExternalInput").ap()
    bq_d = nc.dram_tensor("bq", [D, 1], f32, kind="ExternalInput").ap()
    masks_d = nc.dram_tensor("masks", [2, 128, 1536], bf16, kind="ExternalInput").ap()
    out_d = nc.dram_tensor("out", [ND, NG, VA, 512], f32, kind="ExternalOutput").ap()

    with TileContext(nc) as tc:
        with tc.tile_pool(name="persist", bufs=1) as pp, \
             tc.tile_pool(name="stage", bufs=1) as sp, \
             tc.tile_pool(name="pa512", bufs=3, space="PSUM") as pps, \
             tc.tile_pool(name="probsp", bufs=10) as prp, \
             tc.tile_pool(name="scps", bufs=2, space="PSUM") as scp, \
             tc.tile_pool(name="pvps", bufs=1, space="PSUM") as pvp, \
             tc.tile_pool(name="outp", bufs=3) as outp:
            qT = [pp.tile([128, SQ], bf16, name=f"qT{m}", tag=f"qT{m}") for m in range(ND)]
            kT = [pp.tile([128, SKV], bf16, name=f"kT{m}", tag=f"kT{m}") for m in range(ND)]
            vaug = [pp.tile([128, H * VA], bf16, name=f"vaug{t}", tag=f"vaug{t}") for t in range(NKVT)]

            # ---- staged input tiles; DMAs split into two issue groups:
            # dma_start costs ~0.8us of descriptor generation on the
            # issuing engine, so only the prelude-critical DMAs go first
            # and the rest are emitted after the first compute chunk.
            engines = [nc.sync, nc.scalar, nc.gpsimd]
            valT_t = [sp.tile([128, SKV], bf16, name=f"valTt{k}", tag=f"valTt{k}") for k in range(ND)]
            wv_t = [sp.tile([128, D], bf16, name=f"wvt{k}", tag=f"wvt{k}") for k in range(ND)]
            wq_t = [sp.tile([128, D], bf16, name=f"wqt{k}", tag=f"wqt{k}") for k in range(ND)]
            wk_t = [sp.tile([128, D], bf16, name=f"wkt{k}", tag=f"wkt{k}") for k in range(ND)]
            bqt = [sp.tile([128, 1], f32, name=f"bqt{m}", tag=f"bqt{m}") for m in range(ND)]
            maskt = [sp.tile([128, 1536], bf16, name=f"maskt{i}", tag=f"maskt{i}") for i in range(2)]
            ei = 0
            def dma(dst, src):
                nonlocal ei
                engines[ei % 3].dma_start(dst, src)
                ei += 1

            def emit_dma_group_a():
                # sync+scalar alternate the prelude payloads; gpsimd takes
                # the tiny bq vectors (q(0,0)'s bias evac reads them)
                eng2 = [nc.sync, nc.scalar]
                for k in range(ND):
                    eng2[k % 2].dma_start(wq_t[k][:], wq_d[k * 128:(k + 1) * 128, :])
                    eng2[(k + 1) % 2].dma_start(valT_t[k][:, 0:768], valT_d[k * 128:(k + 1) * 128, 0:768])
                for m in range(ND):
                    nc.gpsimd.dma_start(bqt[m][:], bq_d[m * 128:(m + 1) * 128, :])

            def emit_dma_group_b():
                # sync: valT tails + masks; gpsimd: wk + wv. The scalar
                # queue stays clear: it runs the q-bias/exp activations.
                for k in range(ND):
                    nc.gpsimd.dma_start(wk_t[k][:], wk_d[k * 128:(k + 1) * 128, :])
                    nc.sync.dma_start(valT_t[k][:, 768:SKV], valT_d[k * 128:(k + 1) * 128, 768:SKV])
                for i in range(2):
                    nc.sync.dma_start(maskt[i][:], masks_d[i])
                for k in range(ND):
                    nc.gpsimd.dma_start(wv_t[k][:], wv_d[k * 128:(k + 1) * 128, :])

            for t in range(NKVT):
                ones_col = vaug[t][:].rearrange("p (h c) -> p h c", c=VA)[:, :, HD:VA]
                nc.vector.memset(ones_col, 1.0)

            # ---------------- Phase A chunks (one PSUM group each) --------
            def emit_vproj(t, s):
                ps = pps.tile([128, 512], f32, name="psv", tag="pa")
                lo, hi, h0 = (0, 512, 0) if s == 0 else (512, 768, 8)
                w = hi - lo
                for k in range(ND):
                    nc.tensor.matmul(ps[:, 0:w],
                                     valT_t[k][:, t * 128:(t + 1) * 128],
                                     wv_t[k][:, lo:hi],
                                     start=(k == 0), stop=(k == ND - 1))
                nh = w // HD
                va = vaug[t][:].rearrange("p (h c) -> p h c", c=VA)
                nc.vector.tensor_copy(
                    va[:, h0:h0 + nh, 0:HD],
                    ps[:, 0:w].rearrange("p (h c) -> p h c", c=HD))

            def emit_qproj(m, ch):
                ps = pps.tile([128, 512], f32, name="psq", tag="pa")
                for k in range(ND):
                    nc.tensor.matmul(ps[:],
                                     wq_t[k][:, m * 128:(m + 1) * 128],
                                     valT_t[k][:, W + ch * 512:W + (ch + 1) * 512],
                                     start=(k == 0), stop=(k == ND - 1))
                nc.scalar.activation(qT[m][:, ch * 512:(ch + 1) * 512], ps[:],
                                     AF.Identity, bias=bqt[m][:], scale=1.0)

            def emit_kproj(m, s):
                ps = pps.tile([128, 512], f32, name="psk", tag="pa")
                lo, hi = (s * 512, min((s + 1) * 512, SKV))
                w = hi - lo
                for k in range(ND):
                    nc.tensor.matmul(ps[:, 0:w],
                                     wk_t[k][:, m * 128:(m + 1) * 128],
                                     valT_t[k][:, lo:hi],
                                     start=(k == 0), stop=(k == ND - 1))
                nc.vector.tensor_copy(kT[m][:, lo:hi], ps[:, 0:w])

            # ---------------- Phase B ----------------
            pair_iters = [(mh, g) for mh in range(ND) for g in range(NG)]
            probs_ring = {}

            def emit_qk_side(i):
                mh, g = pair_iters[i]
                pss = [scp.tile([128, 768], f32, name="pss", tag="scores")
                       for _ in range(2)]
                for tau in range(4):
                    for hh in range(2):
                        ph = hh * 64
                        nc.tensor.matmul(
                            pss[hh][:, OFF[tau]:OFF[tau] + WID[tau]],
                            kT[mh][ph:ph + 64, (2 * g + tau) * 128:(2 * g + tau + 1) * 128],
                            qT[mh][ph:ph + 64, g * 256 + QLO[tau]:g * 256 + QLO[tau] + WID[tau]],
                            start=True, stop=True)
                pre = prp.tile([128, 1536], bf16, name="prexp", tag="prexp")
                for hh in range(2):
                    nc.scalar.activation(pre[:, hh * 768:(hh + 1) * 768], pss[hh][:], AF.Exp)
                pr = prp.tile([128, 1536], bf16, name="probs", tag="probs")
                mt = maskt[0] if g == 0 else maskt[1]
                nc.vector.tensor_mul(pr[:], pre[:], mt[:])
                probs_ring[i] = pr

            def emit_pv_side(i):
                mh, g = pair_iters[i]
                pr = probs_ring.pop(i)
                pso = pvp.tile([VA, 512], f32, name="pso", tag="pv")
                for hh in range(2):
                    h = 2 * mh + hh
                    for tau in range(4):
                        nc.tensor.matmul(
                            pso[0:VA, hh * 256 + QLO[tau]:hh * 256 + QLO[tau] + WID[tau]],
                            vaug[2 * g + tau][:, h * VA:(h + 1) * VA],
                            pr[:, hh * 768 + OFF[tau]:hh * 768 + OFF[tau] + WID[tau]],
                            start=(hh == 0 and tau == 0), stop=(hh == 1 and tau == 3))
                ob = outp.tile([VA, 512], f32, name="ob", tag="ob")
                nc.vector.tensor_copy(ob[:], pso[:])
                nc.sync.dma_start(out_d[mh, g], ob[:])

            # ------------- just-in-time interleaved emission -------------
            # tapered PV lag: deep early (so v-projection can trickle in as
            # spacers) and shallow late (short drain tail)
            NPI = len(pair_iters)
            def lag(j):
                return max(2, 8 - j // 2)
            pv_slot = {}
            leftover = []
            for j in range(NPI):
                sl = j + lag(j)
                if sl <= NPI - 1:
                    pv_slot.setdefault(sl, []).append(j)
                else:
                    leftover.append(j)

            # chunks with deadlines (must be emitted before QK/PV slot d)
            chunks = []   # (deadline, emit_fn)
            # q(m, ch0) needs only group-A inputs: pull these early to
            # fill the PE while the group-B DMAs stream in
            QCH0_D = {1: 1, 2: 1, 3: 2, 4: 2, 5: 3}
            for m in range(ND):
                for ch in range(2):
                    if (m, ch) == (0, 0):
                        continue          # prelude
                    d = QCH0_D[m] if ch == 0 else 4 * m + 2
                    chunks.append((d, lambda m=m, ch=ch: emit_qproj(m, ch)))
            for m in range(ND):
                for sn in range(3):
                    d = 4 * m if sn < 2 else 4 * m + 2
                    chunks.append((d, lambda m=m, sn=sn: emit_kproj(m, sn)))
            first_pv_slot = {}
            for slot, js in pv_slot.items():
                for j in js:
                    for t in range(2 * (j % NG), 2 * (j % NG) + 4):
                        if t not in first_pv_slot:
                            first_pv_slot[t] = slot
            for t in range(NKVT):
                d = first_pv_slot.get(t, 8)
                for sn in range(2):
                    chunks.append((d, lambda t=t, sn=sn: emit_vproj(t, sn)))
            chunks.sort(key=lambda c: c[0])

            # prelude: group-A DMAs, first q chunk, then the rest of the
            # input DMAs (k(0,s0/s1) are deadline-0 chunks below)
            emit_dma_group_a()
            emit_qproj(0, 0)
            emit_dma_group_b()

            emitted = 0
            total = len(chunks)
            for i in range(NPI):
                # overdue chunks first
                while emitted < total and chunks[emitted][0] <= i:
                    chunks[emitted][1]()
                    emitted += 1
                emit_qk_side(i)
                # spread remaining chunks evenly across iterations
                target = (i + 1) * total // NPI
                while emitted < min(target, total):
                    chunks[emitted][1]()
                    emitted += 1
                for j in pv_slot.get(i, []):
                    emit_pv_side(j)
            while emitted < total:
                chunks[emitted][1]()
                emitted += 1
            for j in leftover:
                emit_pv_side(j)
    nc.compile()
    return nc


def _get_nc():
    if "nc" not in _CACHE:
        _CACHE["nc"] = _build_nc()
    return _CACHE["nc"]


def _install_ntff_hook():
    """Provide antenv.axon_hooks (absent in this image) so bass_utils can
    NTFF-profile under axon, using trn_agent_boot's ctypes hook builder."""
    import sys
    import types
    try:
        from antenv.axon_hooks import get_axon_ntff_profile_hook  # noqa: F401
        return
    except ImportError:
        pass
    try:
        import antenv
        from trn_agent_boot.trn_boot import _ntff_profile_via_ctypes
        hook = _ntff_profile_via_ctypes("/opt/axon/libaxon_pjrt.so")
        mod = types.ModuleType("antenv.axon_hooks")
        mod.get_axon_ntff_profile_hook = lambda: hook
        mod.set_axon_ntff_profile_hook = lambda h: None
        sys.modules["antenv.axon_hooks"] = mod
        antenv.axon_hooks = mod
    except Exception as e:  # profiling is best-effort
        print(f"ntff hook install failed: {e}")


def kernel(val, Wq, bq, Wk, bk, Wv, bv):
    from concourse.bass_utils import run_bass_kernel_spmd

    val = np.asarray(val, dtype=np.float32)
    Wq = np.asarray(Wq, dtype=np.float32)
    bq = np.asarray(bq, dtype=np.float32)
    Wk = np.asarray(Wk, dtype=np.float32)
    Wv = np.asarray(Wv, dtype=np.float32)
    bv = np.asarray(bv, dtype=np.float32)

    bf = ml_dtypes.bfloat16
    wq_s = np.ascontiguousarray((Wq * SCALE).astype(bf))
    bq_s = np.ascontiguousarray((bq * SCALE).reshape(D, 1))
    wk_c = np.ascontiguousarray(Wk.astype(bf))
    wv_c = np.ascontiguousarray(Wv.astype(bf))

    in_maps = []
    for c in range(NCORES):
        b, qd = divmod(c, 4)
        lo = qd * SQ - W
        hi = qd * SQ + SQ
        vs = val[max(lo, 0):hi, b, :]
        if lo < 0:
            vs = np.concatenate([np.zeros((-lo, D), np.float32), vs], axis=0)
        in_maps.append({
            "valT": np.ascontiguousarray(vs.T.astype(bf)),
            "wq": wq_s, "wk": wk_c, "wv": wv_c, "bq": bq_s,
            "masks": _masks_np(boundary=(qd == 0)),
        })

    nc = _get_nc()
    trace = os.environ.get("BASS_KERNEL_TRACE", "0") == "1"
    kwargs = {}
    if trace:
        _install_ntff_hook()
        kwargs = dict(trace=True, tmpdir=os.environ.get("BASS_KERNEL_TRACE_DIR") or None)
    res = run_bass_kernel_spmd(nc, in_maps, list(range(NCORES)), **kwargs)
    _CACHE["last_result"] = res

    out = np.empty((S, B, D), np.float32)
    for c in range(NCORES):
        b, qd = divmod(c, 4)
        raw = np.asarray(res.results[c]["out"]).reshape(ND, NG, VA, 2, 256)
        core = raw[:, :, 0:HD] / raw[:, :, HD:VA]            # (ND, NG, HD, 2, 256)
        # (mh, g, d, hh, q) -> (g, q, mh, hh, d) -> (SQ, D)
        core = core.transpose(1, 4, 0, 3, 2).reshape(SQ, D)
        out[qd * SQ:(qd + 1) * SQ, b, :] = core
    out += bv
    return out


# revision 17
# speedup vs baseline: 1.1628x; 1.0052x over previous
"""Trainium2 Bass kernel for LocalSelfAttention (sliding-window, causal).

Problem: val (S=4096, B=2, D=768); q/k/v projections then Longformer-style
banded causal attention, window = 256 lookback (keys j in [i-256, i]).

Sharding: 8 cores = batch (2) x sequence quarters (4). Each core handles
1024 queries of one batch element and receives a 256-row key/value halo
(recomputed locally from val rows; no inter-core communication).

Math simplifications (exact up to float rounding):
  - bk dropped: per-query additive constant q.bk cancels in softmax.
  - bv added on host at the end: sum_j p_j (v0_j + bv) = (PV)/Z + bv.
  - no max-subtraction in softmax: scores ~ N(0,1), |s| < ~8, exp is safe.
  - 1/sqrt(hd) folded into Wq/bq on host.
  - final division (PV / rowsum) done on host.

Device pipeline per core (all matmul inputs bf16):
  One fused software-pipelined schedule. A small prelude (q-projection
  feature-tile 0, k-projection tile 0) unblocks attention within ~15us;
  every remaining projection accumulation group (q/k/v per feature- or
  token-tile) is a "chunk" with a deadline, emitted just-in-time between
  attention iterations so the PE stays >90% busy end to end. Input DMAs
  are issued in two groups (prelude payloads first) because each
  dma_start costs ~0.8us of descriptor generation on its engine.

  Attention runs as 24 pair-iterations (6 feature-tiles x 4 query
  groups of 256; a feature-tile holds 2 heads on partition halves
  0-63 / 64-127, and their QK matmuls alternate so they run
  concurrently on disjoint PE row-groups). Scores live in a packed
  [128, 768] PSUM tile holding only the 3-key-tile band (kt1@0,
  kt2@256, kt0@512, kt3@640 - no matmul output straddles a PSUM bank).
  One 768-col EXP per head writes both halves of a [128, 1536] pair
  tile, one DVE multiply applies the band mask for both heads, then PV
  uses stationary = v-tile (65 cols incl. a ones column for row sums)
  and moving = probs, accumulating out[65, 512] per (head-pair, group)
  in a single PSUM bank - the staggered query windows compose via the
  per-element has_written accumulate bits. The PV side trails the QK
  side by a tapered lag (deep early, so v-projection chunks can
  trickle in; shallow late, for a short drain tail). Output tiles are
  DMA'd transposed; the host untransposes and divides by the row sums.
"""

import os
import numpy as np
import ml_dtypes

S, B, D = 4096, 2, 768
H, HD = 12, 64
W = 256
NCORES = 8
SQ = S // 4            # 1024 queries per core
SKV = SQ + W           # 1280 kv rows (halo)
NG = SQ // 256         # 4 query groups of 256
ND = D // 128          # 6 feature tiles
NKVT = SKV // 128      # 10 kv tiles
VA = HD + 1            # 65: per-head v width incl. ones column
SCALE = 1.0 / np.sqrt(HD).astype(np.float32)  # 0.125

# packed scores layout: per 256-query group, 4 key tiles tau=0..3 with
# query windows [QLO, QLO+WID) packed at column OFF (bank-straddle free)
WID = (128, 256, 256, 128)
QLO = (0, 0, 0, 128)
OFF = (512, 0, 256, 640)

_CACHE = {}


def _masks_np(boundary: bool) -> np.ndarray:
    """(2, 128, 768) bf16 band masks in the packed scores layout.

    Partition = key-within-tile pp; columns = packed query windows.
    Set 0 is used for group 0 (kt0/kt1 zeroed on sequence-boundary cores),
    set 1 for groups 1..3.
    """
    pp = np.arange(128)[:, None]
    valid = [None] * 4
    j0 = np.arange(WID[0])[None, :]
    valid[0] = (pp >= j0)                 # keys g*256+pp, queries j
    j1 = np.arange(WID[1])[None, :]
    valid[1] = (j1 <= 128 + pp)
    j2 = np.arange(WID[2])[None, :]
    valid[2] = (j2 >= pp)
    j3 = np.arange(WID[3])[None, :]
    valid[3] = (j3 >= pp)

    def build(zero_lo: bool) -> np.ndarray:
        m = np.zeros((128, 768), np.float32)
        for tau in range(4):
            v = valid[tau].astype(np.float32)
            if zero_lo and tau in (0, 1):
                v = np.zeros_like(v)
            m[:, OFF[tau]:OFF[tau] + WID[tau]] = v
        return m

    m = np.stack([build(boundary), build(False)])
    m = np.concatenate([m, m], axis=2)   # duplicate for the head pair halves
    return np.ascontiguousarray(m.astype(ml_dtypes.bfloat16))


def _build_nc():
    import concourse.bacc as bacc
    import concourse.mybir as mybir
    from concourse.tile import TileContext

    f32 = mybir.dt.float32
    bf16 = mybir.dt.bfloat16
    AF = mybir.ActivationFunctionType

    nc = bacc.Bacc(trn_type="TRN2", debug=False, num_devices=NCORES)

    valT_d = nc.dram_tensor("valT", [D, SKV], bf16, kind="ExternalInput").ap()
    wq_d = nc.dram_tensor("wq", [D, D + 2], bf16, kind="ExternalInput").ap()
    wk_d = nc.dram_tensor("wk", [D, D], bf16, kind="ExternalInput").ap()
    wv_d = nc.dram_tensor("wv", [D, D], bf16, kind="ExternalInput").ap()
    masks_d = nc.dram_tensor("masks", [2, 128, 1536], bf16, kind="ExternalInput").ap()
    out_d = nc.dram_tensor("out", [ND, NG, VA, 512], f32, kind="ExternalOutput").ap()

    with TileContext(nc) as tc:
        with tc.tile_pool(name="persist", bufs=1) as pp, \
             tc.tile_pool(name="stage", bufs=1) as sp, \
             tc.tile_pool(name="pa512", bufs=3, space="PSUM") as pps, \
             tc.tile_pool(name="probsp", bufs=10) as prp, \
             tc.tile_pool(name="scps", bufs=2, space="PSUM") as scp, \
             tc.tile_pool(name="pvps", bufs=1, space="PSUM") as pvp, \
             tc.tile_pool(name="outp", bufs=3) as outp:
            qT = [pp.tile([128, SQ], bf16, name=f"qT{m}", tag=f"qT{m}") for m in range(ND)]
            kT = [pp.tile([128, SKV], bf16, name=f"kT{m}", tag=f"kT{m}") for m in range(ND)]
            vaug = [pp.tile([128, H * VA], bf16, name=f"vaug{t}", tag=f"vaug{t}") for t in range(NKVT)]

            # ---- staged input tiles; DMAs split into two issue groups:
            # dma_start costs ~0.8us of descriptor generation on the
            # issuing engine, so only the prelude-critical DMAs go first
            # and the rest are emitted after the first compute chunk.
            engines = [nc.sync, nc.scalar, nc.gpsimd]
            valT_t = [sp.tile([128, SKV], bf16, name=f"valTt{k}", tag=f"valTt{k}") for k in range(ND)]
            wv_t = [sp.tile([128, D], bf16, name=f"wvt{k}", tag=f"wvt{k}") for k in range(ND)]
            wq_t = [sp.tile([128, D + 2], bf16, name=f"wqt{k}", tag=f"wqt{k}") for k in range(ND)]
            wk_t = [sp.tile([128, D], bf16, name=f"wkt{k}", tag=f"wkt{k}") for k in range(ND)]
            bqt = [wq_t[m][:, D:D + 1] for m in range(ND)]   # bias rides in wq col 768
            maskt = [sp.tile([128, 1536], bf16, name=f"maskt{i}", tag=f"maskt{i}") for i in range(2)]
            ei = 0
            def dma(dst, src):
                nonlocal ei
                engines[ei % 3].dma_start(dst, src)
                ei += 1

            def emit_dma_group_a():
                for k in range(ND):
                    dma(wq_t[k][:], wq_d[k * 128:(k + 1) * 128, :])
                    dma(valT_t[k][:, 0:768], valT_d[k * 128:(k + 1) * 128, 0:768])

            def emit_dma_group_b():
                # gpsimd: wk + masks + wv; sync: valT tails. The scalar
                # queue stays clear: it runs the q-bias/exp activations.
                for k in range(ND):
                    nc.gpsimd.dma_start(wk_t[k][:], wk_d[k * 128:(k + 1) * 128, :])
                    nc.sync.dma_start(valT_t[k][:, 768:SKV], valT_d[k * 128:(k + 1) * 128, 768:SKV])
                for i in range(2):
                    nc.gpsimd.dma_start(maskt[i][:], masks_d[i])
                for k in range(ND):
                    nc.gpsimd.dma_start(wv_t[k][:], wv_d[k * 128:(k + 1) * 128, :])

            for t in range(NKVT):
                ones_col = vaug[t][:].rearrange("p (h c) -> p h c", c=VA)[:, :, HD:VA]
                nc.vector.memset(ones_col, 1.0)

            # ---------------- Phase A chunks (one PSUM group each) --------
            def emit_vproj(t, s):
                ps = pps.tile([128, 512], f32, name="psv", tag="pa")
                lo, hi, h0 = (0, 512, 0) if s == 0 else (512, 768, 8)
                w = hi - lo
                for k in range(ND):
                    nc.tensor.matmul(ps[:, 0:w],
                                     valT_t[k][:, t * 128:(t + 1) * 128],
                                     wv_t[k][:, lo:hi],
                                     start=(k == 0), stop=(k == ND - 1))
                nh = w // HD
                va = vaug[t][:].rearrange("p (h c) -> p h c", c=VA)
                nc.vector.tensor_copy(
                    va[:, h0:h0 + nh, 0:HD],
                    ps[:, 0:w].rearrange("p (h c) -> p h c", c=HD))

            def emit_qproj(m, ch):
                ps = pps.tile([128, 512], f32, name="psq", tag="pa")
                for k in range(ND):
                    nc.tensor.matmul(ps[:],
                                     wq_t[k][:, m * 128:(m + 1) * 128],
                                     valT_t[k][:, W + ch * 512:W + (ch + 1) * 512],
                                     start=(k == 0), stop=(k == ND - 1))
                nc.scalar.activation(qT[m][:, ch * 512:(ch + 1) * 512], ps[:],
                                     AF.Identity, bias=bqt[m], scale=1.0)

            def emit_kproj(m, s):
                ps = pps.tile([128, 512], f32, name="psk", tag="pa")
                lo, hi = (s * 512, min((s + 1) * 512, SKV))
                w = hi - lo
                for k in range(ND):
                    nc.tensor.matmul(ps[:, 0:w],
                                     wk_t[k][:, m * 128:(m + 1) * 128],
                                     valT_t[k][:, lo:hi],
                                     start=(k == 0), stop=(k == ND - 1))
                nc.vector.tensor_copy(kT[m][:, lo:hi], ps[:, 0:w])

            # ---------------- Phase B ----------------
            pair_iters = [(mh, g) for mh in range(ND) for g in range(NG)]
            probs_ring = {}

            def emit_qk_side(i):
                mh, g = pair_iters[i]
                pss = [scp.tile([128, 768], f32, name="pss", tag="scores")
                       for _ in range(2)]
                for tau in range(4):
                    for hh in range(2):
                        ph = hh * 64
                        nc.tensor.matmul(
                            pss[hh][:, OFF[tau]:OFF[tau] + WID[tau]],
                            kT[mh][ph:ph + 64, (2 * g + tau) * 128:(2 * g + tau + 1) * 128],
                            qT[mh][ph:ph + 64, g * 256 + QLO[tau]:g * 256 + QLO[tau] + WID[tau]],
                            start=True, stop=True)
                pre = prp.tile([128, 1536], bf16, name="prexp", tag="prexp")
                for hh in range(2):
                    nc.scalar.activation(pre[:, hh * 768:(hh + 1) * 768], pss[hh][:], AF.Exp)
                pr = prp.tile([128, 1536], bf16, name="probs", tag="probs")
                mt = maskt[0] if g == 0 else maskt[1]
                nc.vector.tensor_mul(pr[:], pre[:], mt[:])
                probs_ring[i] = pr

            def emit_pv_side(i):
                mh, g = pair_iters[i]
                pr = probs_ring.pop(i)
                pso = pvp.tile([VA, 512], f32, name="pso", tag="pv")
                for hh in range(2):
                    h = 2 * mh + hh
                    for tau in range(4):
                        nc.tensor.matmul(
                            pso[0:VA, hh * 256 + QLO[tau]:hh * 256 + QLO[tau] + WID[tau]],
                            vaug[2 * g + tau][:, h * VA:(h + 1) * VA],
                            pr[:, hh * 768 + OFF[tau]:hh * 768 + OFF[tau] + WID[tau]],
                            start=(hh == 0 and tau == 0), stop=(hh == 1 and tau == 3))
                ob = outp.tile([VA, 512], f32, name="ob", tag="ob")
                nc.vector.tensor_copy(ob[:], pso[:])
                nc.sync.dma_start(out_d[mh, g], ob[:])

            # ------------- just-in-time interleaved emission -------------
            # tapered PV lag: deep early (so v-projection can trickle in as
            # spacers) and shallow late (short drain tail)
            NPI = len(pair_iters)
            def lag(j):
                return max(2, 8 - j // 2)
            pv_slot = {}
            leftover = []
            for j in range(NPI):
                sl = j + lag(j)
                if sl <= NPI - 1:
                    pv_slot.setdefault(sl, []).append(j)
                else:
                    leftover.append(j)

            # chunks with deadlines (must be emitted before QK/PV slot d)
            chunks = []   # (deadline, emit_fn)
            # q(m, ch0) needs only group-A inputs: pull these early to
            # fill the PE while the group-B DMAs stream in
            QCH0_D = {1: 1, 2: 1, 3: 2, 4: 2, 5: 3}
            for m in range(ND):
                for ch in range(2):
                    if (m, ch) == (0, 0):
                        continue          # prelude
                    d = QCH0_D[m] if ch == 0 else 4 * m + 2
                    chunks.append((d, lambda m=m, ch=ch: emit_qproj(m, ch)))
            for m in range(ND):
                for sn in range(3):
                    d = 4 * m if sn < 2 else 4 * m + 2
                    chunks.append((d, lambda m=m, sn=sn: emit_kproj(m, sn)))
            first_pv_slot = {}
            for slot, js in pv_slot.items():
                for j in js:
                    for t in range(2 * (j % NG), 2 * (j % NG) + 4):
                        if t not in first_pv_slot:
                            first_pv_slot[t] = slot
            for t in range(NKVT):
                d = first_pv_slot.get(t, 8)
                for sn in range(2):
                    chunks.append((d, lambda t=t, sn=sn: emit_vproj(t, sn)))
            chunks.sort(key=lambda c: c[0])

            # prelude: group-A DMAs, first q chunk, then the rest of the
            # input DMAs (k(0,s0/s1) are deadline-0 chunks below)
            emit_dma_group_a()
            emit_qproj(0, 0)
            emit_dma_group_b()

            emitted = 0
            total = len(chunks)
            for i in range(NPI):
                # overdue chunks first
                while emitted < total and chunks[emitted][0] <= i:
                    chunks[emitted][1]()
                    emitted += 1
                emit_qk_side(i)
                # spread remaining chunks evenly across iterations
                target = (i + 1) * total // NPI
                while emitted < min(target, total):
                    chunks[emitted][1]()
                    emitted += 1
                for j in pv_slot.get(i, []):
                    emit_pv_side(j)
            while emitted < total:
                chunks[emitted][1]()
                emitted += 1
            for j in leftover:
                emit_pv_side(j)
    nc.compile()
    return nc


def _get_nc():
    if "nc" not in _CACHE:
        _CACHE["nc"] = _build_nc()
    return _CACHE["nc"]


def _install_ntff_hook():
    """Provide antenv.axon_hooks (absent in this image) so bass_utils can
    NTFF-profile under axon, using trn_agent_boot's ctypes hook builder."""
    import sys
    import types
    try:
        from antenv.axon_hooks import get_axon_ntff_profile_hook  # noqa: F401
        return
    except ImportError:
        pass
    try:
        import antenv
        from trn_agent_boot.trn_boot import _ntff_profile_via_ctypes
        hook = _ntff_profile_via_ctypes("/opt/axon/libaxon_pjrt.so")
        mod = types.ModuleType("antenv.axon_hooks")
        mod.get_axon_ntff_profile_hook = lambda: hook
        mod.set_axon_ntff_profile_hook = lambda h: None
        sys.modules["antenv.axon_hooks"] = mod
        antenv.axon_hooks = mod
    except Exception as e:  # profiling is best-effort
        print(f"ntff hook install failed: {e}")


def kernel(val, Wq, bq, Wk, bk, Wv, bv):
    from concourse.bass_utils import run_bass_kernel_spmd

    val = np.asarray(val, dtype=np.float32)
    Wq = np.asarray(Wq, dtype=np.float32)
    bq = np.asarray(bq, dtype=np.float32)
    Wk = np.asarray(Wk, dtype=np.float32)
    Wv = np.asarray(Wv, dtype=np.float32)
    bv = np.asarray(bv, dtype=np.float32)

    bf = ml_dtypes.bfloat16
    wq_s = np.zeros((D, D + 2), dtype=bf)
    wq_s[:, 0:D] = (Wq * SCALE).astype(bf)
    wq_s[:, D] = (bq * SCALE).astype(bf)
    wq_s = np.ascontiguousarray(wq_s)
    wk_c = np.ascontiguousarray(Wk.astype(bf))
    wv_c = np.ascontiguousarray(Wv.astype(bf))

    in_maps = []
    for c in range(NCORES):
        b, qd = divmod(c, 4)
        lo = qd * SQ - W
        hi = qd * SQ + SQ
        vs = val[max(lo, 0):hi, b, :]
        if lo < 0:
            vs = np.concatenate([np.zeros((-lo, D), np.float32), vs], axis=0)
        in_maps.append({
            "valT": np.ascontiguousarray(vs.T.astype(bf)),
            "wq": wq_s, "wk": wk_c, "wv": wv_c,
            "masks": _masks_np(boundary=(qd == 0)),
        })

    nc = _get_nc()
    trace = os.environ.get("BASS_KERNEL_TRACE", "0") == "1"
    kwargs = {}
    if trace:
        _install_ntff_hook()
        kwargs = dict(trace=True, tmpdir=os.environ.get("BASS_KERNEL_TRACE_DIR") or None)
    res = run_bass_kernel_spmd(nc, in_maps, list(range(NCORES)), **kwargs)
    _CACHE["last_result"] = res

    out = np.empty((S, B, D), np.float32)
    for c in range(NCORES):
        b, qd = divmod(c, 4)
        raw = np.asarray(res.results[c]["out"]).reshape(ND, NG, VA, 2, 256)
        core = raw[:, :, 0:HD] / raw[:, :, HD:VA]            # (ND, NG, HD, 2, 256)
        # (mh, g, d, hh, q) -> (g, q, mh, hh, d) -> (SQ, D)
        core = core.transpose(1, 4, 0, 3, 2).reshape(SQ, D)
        out[qd * SQ:(qd + 1) * SQ, b, :] = core
    out += bv
    return out


# revision 18
# speedup vs baseline: 1.1678x; 1.0043x over previous
"""Trainium2 Bass kernel for LocalSelfAttention (sliding-window, causal).

Problem: val (S=4096, B=2, D=768); q/k/v projections then Longformer-style
banded causal attention, window = 256 lookback (keys j in [i-256, i]).

Sharding: 8 cores = batch (2) x sequence quarters (4). Each core handles
1024 queries of one batch element and receives a 256-row key/value halo
(recomputed locally from val rows; no inter-core communication).

Math simplifications (exact up to float rounding):
  - bk dropped: per-query additive constant q.bk cancels in softmax.
  - bv added on host at the end: sum_j p_j (v0_j + bv) = (PV)/Z + bv.
  - no max-subtraction in softmax: scores ~ N(0,1), |s| < ~8, exp is safe.
  - 1/sqrt(hd) folded into Wq/bq on host.
  - final division (PV / rowsum) done on host.

Device pipeline per core (all matmul inputs bf16):
  One fused software-pipelined schedule. A small prelude (q-projection
  feature-tile 0, k-projection tile 0) unblocks attention within ~15us;
  every remaining projection accumulation group (q/k/v per feature- or
  token-tile) is a "chunk" with a deadline, emitted just-in-time between
  attention iterations so the PE stays >90% busy end to end. Input DMAs
  are issued in two groups (prelude payloads first) because each
  dma_start costs ~0.8us of descriptor generation on its engine; the
  q bias vector rides in an extra wq column (tile k=m holds rows
  m*128..m*128+127, so column 768 of tile m is exactly bias slice m),
  saving six more DMA issues.

  Attention runs as 24 pair-iterations (6 feature-tiles x 4 query
  groups of 256; a feature-tile holds 2 heads on partition halves
  0-63 / 64-127, and their QK matmuls alternate so they run
  concurrently on disjoint PE row-groups). Scores live in a packed
  [128, 768] PSUM tile holding only the 3-key-tile band (kt1@0,
  kt2@256, kt0@512, kt3@640 - no matmul output straddles a PSUM bank).
  One 768-col EXP per head writes both halves of a [128, 1536] pair
  tile, one DVE multiply applies the band mask for both heads, then PV
  uses stationary = v-tile (65 cols incl. a ones column for row sums)
  and moving = probs, accumulating out[65, 512] per (head-pair, group)
  in a single PSUM bank - the staggered query windows compose via the
  per-element has_written accumulate bits. The PV side trails the QK
  side by a tapered lag (deep early, so v-projection chunks can
  trickle in; shallow late, for a short drain tail). Output tiles are
  DMA'd transposed; the host untransposes and divides by the row sums.
"""

import os
import numpy as np
import ml_dtypes

S, B, D = 4096, 2, 768
H, HD = 12, 64
W = 256
NCORES = 8
SQ = S // 4            # 1024 queries per core
SKV = SQ + W           # 1280 kv rows (halo)
NG = SQ // 256         # 4 query groups of 256
ND = D // 128          # 6 feature tiles
NKVT = SKV // 128      # 10 kv tiles
VA = HD + 1            # 65: per-head v width incl. ones column
SCALE = 1.0 / np.sqrt(HD).astype(np.float32)  # 0.125

# packed scores layout: per 256-query group, 4 key tiles tau=0..3 with
# query windows [QLO, QLO+WID) packed at column OFF (bank-straddle free)
WID = (128, 256, 256, 128)
QLO = (0, 0, 0, 128)
OFF = (512, 0, 256, 640)

_CACHE = {}


def _masks_np(boundary: bool) -> np.ndarray:
    """(2, 128, 768) bf16 band masks in the packed scores layout.

    Partition = key-within-tile pp; columns = packed query windows.
    Set 0 is used for group 0 (kt0/kt1 zeroed on sequence-boundary cores),
    set 1 for groups 1..3.
    """
    pp = np.arange(128)[:, None]
    valid = [None] * 4
    j0 = np.arange(WID[0])[None, :]
    valid[0] = (pp >= j0)                 # keys g*256+pp, queries j
    j1 = np.arange(WID[1])[None, :]
    valid[1] = (j1 <= 128 + pp)
    j2 = np.arange(WID[2])[None, :]
    valid[2] = (j2 >= pp)
    j3 = np.arange(WID[3])[None, :]
    valid[3] = (j3 >= pp)

    def build(zero_lo: bool) -> np.ndarray:
        m = np.zeros((128, 768), np.float32)
        for tau in range(4):
            v = valid[tau].astype(np.float32)
            if zero_lo and tau in (0, 1):
                v = np.zeros_like(v)
            m[:, OFF[tau]:OFF[tau] + WID[tau]] = v
        return m

    m = np.stack([build(boundary), build(False)])
    m = np.concatenate([m, m], axis=2)   # duplicate for the head pair halves
    return np.ascontiguousarray(m.astype(ml_dtypes.bfloat16))


def _build_nc():
    import concourse.bacc as bacc
    import concourse.mybir as mybir
    from concourse.tile import TileContext

    f32 = mybir.dt.float32
    bf16 = mybir.dt.bfloat16
    AF = mybir.ActivationFunctionType

    nc = bacc.Bacc(trn_type="TRN2", debug=False, num_devices=NCORES)

    valT_d = nc.dram_tensor("valT", [D, SKV], bf16, kind="ExternalInput").ap()
    wq_d = nc.dram_tensor("wq", [D, D + 2], bf16, kind="ExternalInput").ap()
    wk_d = nc.dram_tensor("wk", [D, D], bf16, kind="ExternalInput").ap()
    wv_d = nc.dram_tensor("wv", [D, D], bf16, kind="ExternalInput").ap()
    masks_d = nc.dram_tensor("masks", [2, 128, 1536], bf16, kind="ExternalInput").ap()
    out_d = nc.dram_tensor("out", [ND, NG, VA, 512], f32, kind="ExternalOutput").ap()

    with TileContext(nc) as tc:
        with tc.tile_pool(name="persist", bufs=1) as pp, \
             tc.tile_pool(name="stage", bufs=1) as sp, \
             tc.tile_pool(name="pa512", bufs=3, space="PSUM") as pps, \
             tc.tile_pool(name="probsp", bufs=10) as prp, \
             tc.tile_pool(name="scps", bufs=2, space="PSUM") as scp, \
             tc.tile_pool(name="pvps", bufs=1, space="PSUM") as pvp, \
             tc.tile_pool(name="outp", bufs=3) as outp:
            qT = [pp.tile([128, SQ], bf16, name=f"qT{m}", tag=f"qT{m}") for m in range(ND)]
            kT = [pp.tile([128, SKV], bf16, name=f"kT{m}", tag=f"kT{m}") for m in range(ND)]
            vaug = [pp.tile([128, H * VA], bf16, name=f"vaug{t}", tag=f"vaug{t}") for t in range(NKVT)]

            # ---- staged input tiles; DMAs split into two issue groups:
            # dma_start costs ~0.8us of descriptor generation on the
            # issuing engine, so only the prelude-critical DMAs go first
            # and the rest are emitted after the first compute chunk.
            engines = [nc.sync, nc.scalar, nc.gpsimd]
            valT_t = [sp.tile([128, SKV], bf16, name=f"valTt{k}", tag=f"valTt{k}") for k in range(ND)]
            wv_t = [sp.tile([128, D], bf16, name=f"wvt{k}", tag=f"wvt{k}") for k in range(ND)]
            wq_t = [sp.tile([128, D + 2], bf16, name=f"wqt{k}", tag=f"wqt{k}") for k in range(ND)]
            wk_t = [sp.tile([128, D], bf16, name=f"wkt{k}", tag=f"wkt{k}") for k in range(ND)]
            bqt = [wq_t[m][:, D:D + 1] for m in range(ND)]   # bias rides in wq col 768
            maskt = [sp.tile([128, 1536], bf16, name=f"maskt{i}", tag=f"maskt{i}") for i in range(2)]
            ei = 0
            def dma(dst, src):
                nonlocal ei
                engines[ei % 3].dma_start(dst, src)
                ei += 1

            def emit_dma_group_a():
                for k in range(ND):
                    dma(wq_t[k][:], wq_d[k * 128:(k + 1) * 128, :])
                    dma(valT_t[k][:, 0:768], valT_d[k * 128:(k + 1) * 128, 0:768])

            def emit_dma_group_b():
                # gpsimd: wk + masks + wv; sync: valT tails. The scalar
                # queue stays clear: it runs the q-bias/exp activations.
                for k in range(ND):
                    nc.gpsimd.dma_start(wk_t[k][:], wk_d[k * 128:(k + 1) * 128, :])
                    nc.sync.dma_start(valT_t[k][:, 768:SKV], valT_d[k * 128:(k + 1) * 128, 768:SKV])
                for i in range(2):
                    nc.gpsimd.dma_start(maskt[i][:], masks_d[i])
                for k in range(ND):
                    nc.gpsimd.dma_start(wv_t[k][:], wv_d[k * 128:(k + 1) * 128, :])

            for t in range(NKVT):
                ones_col = vaug[t][:].rearrange("p (h c) -> p h c", c=VA)[:, :, HD:VA]
                nc.vector.memset(ones_col, 1.0)

            # ---------------- Phase A chunks (one PSUM group each) --------
            def emit_vproj(t, s):
                ps = pps.tile([128, 512], f32, name="psv", tag="pa")
                lo, hi, h0 = (0, 512, 0) if s == 0 else (512, 768, 8)
                w = hi - lo
                for k in range(ND):
                    nc.tensor.matmul(ps[:, 0:w],
                                     valT_t[k][:, t * 128:(t + 1) * 128],
                                     wv_t[k][:, lo:hi],
                                     start=(k == 0), stop=(k == ND - 1))
                nh = w // HD
                va = vaug[t][:].rearrange("p (h c) -> p h c", c=VA)
                nc.vector.tensor_copy(
                    va[:, h0:h0 + nh, 0:HD],
                    ps[:, 0:w].rearrange("p (h c) -> p h c", c=HD))

            def emit_qproj(m, ch):
                ps = pps.tile([128, 512], f32, name="psq", tag="pa")
                for k in range(ND):
                    nc.tensor.matmul(ps[:],
                                     wq_t[k][:, m * 128:(m + 1) * 128],
                                     valT_t[k][:, W + ch * 512:W + (ch + 1) * 512],
                                     start=(k == 0), stop=(k == ND - 1))
                nc.scalar.activation(qT[m][:, ch * 512:(ch + 1) * 512], ps[:],
                                     AF.Identity, bias=bqt[m], scale=1.0)

            def emit_kproj(m, s):
                ps = pps.tile([128, 512], f32, name="psk", tag="pa")
                lo, hi = (s * 512, min((s + 1) * 512, SKV))
                w = hi - lo
                for k in range(ND):
                    nc.tensor.matmul(ps[:, 0:w],
                                     wk_t[k][:, m * 128:(m + 1) * 128],
                                     valT_t[k][:, lo:hi],
                                     start=(k == 0), stop=(k == ND - 1))
                nc.vector.tensor_copy(kT[m][:, lo:hi], ps[:, 0:w])

            # ---------------- Phase B ----------------
            pair_iters = [(mh, g) for mh in range(ND) for g in range(NG)]
            probs_ring = {}

            def emit_qk_side(i):
                mh, g = pair_iters[i]
                pss = [scp.tile([128, 768], f32, name="pss", tag="scores")
                       for _ in range(2)]
                for tau in range(4):
                    for hh in range(2):
                        ph = hh * 64
                        nc.tensor.matmul(
                            pss[hh][:, OFF[tau]:OFF[tau] + WID[tau]],
                            kT[mh][ph:ph + 64, (2 * g + tau) * 128:(2 * g + tau + 1) * 128],
                            qT[mh][ph:ph + 64, g * 256 + QLO[tau]:g * 256 + QLO[tau] + WID[tau]],
                            start=True, stop=True)
                pre = prp.tile([128, 1536], bf16, name="prexp", tag="prexp")
                for hh in range(2):
                    nc.scalar.activation(pre[:, hh * 768:(hh + 1) * 768], pss[hh][:], AF.Exp)
                pr = prp.tile([128, 1536], bf16, name="probs", tag="probs")
                mt = maskt[0] if g == 0 else maskt[1]
                nc.vector.tensor_mul(pr[:], pre[:], mt[:])
                probs_ring[i] = pr

            def emit_pv_side(i):
                mh, g = pair_iters[i]
                pr = probs_ring.pop(i)
                pso = pvp.tile([VA, 512], f32, name="pso", tag="pv")
                for hh in range(2):
                    h = 2 * mh + hh
                    for tau in range(4):
                        nc.tensor.matmul(
                            pso[0:VA, hh * 256 + QLO[tau]:hh * 256 + QLO[tau] + WID[tau]],
                            vaug[2 * g + tau][:, h * VA:(h + 1) * VA],
                            pr[:, hh * 768 + OFF[tau]:hh * 768 + OFF[tau] + WID[tau]],
                            start=(hh == 0 and tau == 0), stop=(hh == 1 and tau == 3))
                ob = outp.tile([VA, 512], f32, name="ob", tag="ob")
                nc.vector.tensor_copy(ob[:], pso[:])
                nc.sync.dma_start(out_d[mh, g], ob[:])

            # ------------- just-in-time interleaved emission -------------
            # tapered PV lag: deep early (so v-projection can trickle in as
            # spacers) and shallow late (short drain tail)
            NPI = len(pair_iters)
            def lag(j):
                return max(2, 8 - j // 2)
            pv_slot = {}
            leftover = []
            for j in range(NPI):
                sl = j + lag(j)
                if sl <= NPI - 1:
                    pv_slot.setdefault(sl, []).append(j)
                else:
                    leftover.append(j)

            # chunks with deadlines (must be emitted before QK/PV slot d)
            chunks = []   # (deadline, emit_fn)
            # q(m, ch0) needs only group-A inputs: pull these early to
            # fill the PE while the group-B DMAs stream in
            QCH0_D = {1: 1, 2: 1, 3: 2, 4: 2, 5: 3}
            for m in range(ND):
                for ch in range(2):
                    if (m, ch) == (0, 0):
                        continue          # prelude
                    d = QCH0_D[m] if ch == 0 else 4 * m + 2
                    chunks.append((d, lambda m=m, ch=ch: emit_qproj(m, ch)))
            for m in range(ND):
                for sn in range(3):
                    d = 4 * m if sn < 2 else 4 * m + 2
                    chunks.append((d, lambda m=m, sn=sn: emit_kproj(m, sn)))
            first_pv_slot = {}
            for slot, js in pv_slot.items():
                for j in js:
                    for t in range(2 * (j % NG), 2 * (j % NG) + 4):
                        if t not in first_pv_slot:
                            first_pv_slot[t] = slot
            for t in range(NKVT):
                d = first_pv_slot.get(t, 8)
                for sn in range(2):
                    chunks.append((d, lambda t=t, sn=sn: emit_vproj(t, sn)))
            chunks.sort(key=lambda c: c[0])

            # prelude: group-A DMAs, first q chunk, then the rest of the
            # input DMAs (k(0,s0/s1) are deadline-0 chunks below)
            emit_dma_group_a()
            emit_qproj(0, 0)
            emit_dma_group_b()

            emitted = 0
            total = len(chunks)
            for i in range(NPI):
                # overdue chunks first
                while emitted < total and chunks[emitted][0] <= i:
                    chunks[emitted][1]()
                    emitted += 1
                emit_qk_side(i)
                # spread remaining chunks evenly across iterations
                target = (i + 1) * total // NPI
                while emitted < min(target, total):
                    chunks[emitted][1]()
                    emitted += 1
                for j in pv_slot.get(i, []):
                    emit_pv_side(j)
            while emitted < total:
                chunks[emitted][1]()
                emitted += 1
            for j in leftover:
                emit_pv_side(j)
    nc.compile()
    return nc


def _get_nc():
    if "nc" not in _CACHE:
        _CACHE["nc"] = _build_nc()
    return _CACHE["nc"]


def _install_ntff_hook():
    """Provide antenv.axon_hooks (absent in this image) so bass_utils can
    NTFF-profile under axon, using trn_agent_boot's ctypes hook builder."""
    import sys
    import types
    try:
        from antenv.axon_hooks import get_axon_ntff_profile_hook  # noqa: F401
        return
    except ImportError:
        pass
    try:
        import antenv
        from trn_agent_boot.trn_boot import _ntff_profile_via_ctypes
        hook = _ntff_profile_via_ctypes("/opt/axon/libaxon_pjrt.so")
        mod = types.ModuleType("antenv.axon_hooks")
        mod.get_axon_ntff_profile_hook = lambda: hook
        mod.set_axon_ntff_profile_hook = lambda h: None
        sys.modules["antenv.axon_hooks"] = mod
        antenv.axon_hooks = mod
    except Exception as e:  # profiling is best-effort
        print(f"ntff hook install failed: {e}")


def kernel(val, Wq, bq, Wk, bk, Wv, bv):
    from concourse.bass_utils import run_bass_kernel_spmd

    val = np.asarray(val, dtype=np.float32)
    Wq = np.asarray(Wq, dtype=np.float32)
    bq = np.asarray(bq, dtype=np.float32)
    Wk = np.asarray(Wk, dtype=np.float32)
    Wv = np.asarray(Wv, dtype=np.float32)
    bv = np.asarray(bv, dtype=np.float32)

    bf = ml_dtypes.bfloat16
    wq_s = np.zeros((D, D + 2), dtype=bf)
    wq_s[:, 0:D] = (Wq * SCALE).astype(bf)
    wq_s[:, D] = (bq * SCALE).astype(bf)
    wq_s = np.ascontiguousarray(wq_s)
    wk_c = np.ascontiguousarray(Wk.astype(bf))
    wv_c = np.ascontiguousarray(Wv.astype(bf))

    in_maps = []
    for c in range(NCORES):
        b, qd = divmod(c, 4)
        lo = qd * SQ - W
        hi = qd * SQ + SQ
        vs = val[max(lo, 0):hi, b, :]
        if lo < 0:
            vs = np.concatenate([np.zeros((-lo, D), np.float32), vs], axis=0)
        in_maps.append({
            "valT": np.ascontiguousarray(vs.T.astype(bf)),
            "wq": wq_s, "wk": wk_c, "wv": wv_c,
            "masks": _masks_np(boundary=(qd == 0)),
        })

    nc = _get_nc()
    trace = os.environ.get("BASS_KERNEL_TRACE", "0") == "1"
    kwargs = {}
    if trace:
        _install_ntff_hook()
        kwargs = dict(trace=True, tmpdir=os.environ.get("BASS_KERNEL_TRACE_DIR") or None)
    res = run_bass_kernel_spmd(nc, in_maps, list(range(NCORES)), **kwargs)
    _CACHE["last_result"] = res

    out = np.empty((S, B, D), np.float32)
    for c in range(NCORES):
        b, qd = divmod(c, 4)
        raw = np.asarray(res.results[c]["out"]).reshape(ND, NG, VA, 2, 256)
        core = raw[:, :, 0:HD] / raw[:, :, HD:VA]            # (ND, NG, HD, 2, 256)
        # (mh, g, d, hh, q) -> (g, q, mh, hh, d) -> (SQ, D)
        core = core.transpose(1, 4, 0, 3, 2).reshape(SQ, D)
        out[qd * SQ:(qd + 1) * SQ, b, :] = core
    out += bv
    return out
